# revision 5
# baseline (speedup 1.0000x reference)
"""DilatedAttention Trainium2 kernel (8 NeuronCores, SPMD).

Input  : q, k, v each (2, 24, 8192, 64) float32.
Output : same shape; per head-group windowed attention over dilated
         positions, non-dilated positions zero.

Sharding: 3 head groups x (b in 2, hg in 8) = 16 (b,head) pairs per
group. Core c takes pairs {2c, 2c+1} of every group -> 6 slices per
core, perfectly balanced, no cross-core communication.

Per-core kernel: for each slice, process segments in "quads" (8
segments = 4 duos). A duo packs 2 segments on partition halves:
 - Q/K loaded per 2-quad block (16 segs) as [128, 512] f32 with the
   first 8 segs on partitions 0:64 and the next 8 on 64:128 (spreads
   DMA descriptors over all 16 SDMA engines, 1 HWDGE call each).
 - V loaded duo-stacked [128, 4*65] with a ones column, 1 SWDGE call
   per quad spanning both partition halves.
 - one PE transpose per 128-col chunk yields Q^T/K^T duo-stacked
   [128, m] (B-half chunks transpose from partitions 64:128 via an
   identity replicated on both partition halves).
 - mm1 per half: lt[k,q] = K^T.T @ Q^T   (contraction d=64)
 - exp on ACT (PSUM->SBUF bf16, scale=1/sqrt(d); no max-subtraction
   needed: logits are O(5))
 - mm2 per half: [out_un | s] = e.T @ [V | 1]  (contraction k=m)
 - reciprocal + per-partition broadcast scale on DVE, single merged
   HWDGE store of the dilated rows (ExternalOutput pre-zeroed).

All PSUM tiles are full-bank sized: sub-bank PSUM tiles get packed at
non-bank-aligned offsets, and a matmul output that crosses a PSUM bank
boundary is fatal on hardware.
"""

import sys

if "/opt/trn_rl_repo" not in sys.path:
    sys.path.insert(0, "/opt/trn_rl_repo")

from contextlib import ExitStack

import numpy as np

import concourse.bass as bass  # noqa: F401
import concourse.mybir as mybir
import concourse.tile as tile
from concourse import bacc
from concourse.bass_utils import run_bass_kernel_spmd

B, H, S, D = 2, 24, 8192, 64
W_LIST = [64, 128, 256]
R_LIST = [1, 2, 4]
NG = 3
G = H // NG  # heads per group
N_CORES = 8
SCALE = 1.0 / (D**0.5)

# slice order per core: (group, pair_within_core)
SLICES = [(0, 0), (0, 1), (1, 0), (1, 1), (2, 0), (2, 1)]

F32 = mybir.dt.float32
BF16 = mybir.dt.bfloat16

_PROGRAM = None
LAST_RESULT = None  # BassKernelResults of the most recent run (for test.py)


def build_slice(nc, tc, pools, identb, qs_ap, ks_ap, vs_ap, os_ap, g, nq=None):
    """Emit the program for one (b, head) slice of group g.

    qs_ap.. are [S, D] DRAM APs for this slice.
    """
    qk, ld, tp, sb, ps, outp = pools
    w, r = W_LIST[g], R_LIST[g]
    off = g * r
    m = len(range(off, w, r))  # 64 / 63 / 62
    mp = m + (m & 1)  # even column pitch (PSUM bf16 needs 4B alignment)
    n = qs_ap.shape[0] // w
    if nq is None:
        nq = n // 8
    # partition ranges of the two duo halves; one fused range when m == 64
    halves = [(0, 128)] if m == 64 else [(0, m), (64, 64 + m)]

    qv = qs_ap.rearrange("(n w) d -> w n d", w=w)[off::r]
    kv = ks_ap.rearrange("(n w) d -> w n d", w=w)[off::r]
    vv = vs_ap.rearrange("(n w) d -> w n d", w=w)[off::r]
    ov = os_ap.rearrange("(n w) d -> w n d", w=w)[off::r]

    qd = kd = vd = vb = None
    for t in range(nq):
        s0 = 8 * t
        qh = t & 1  # which partition half of the 2-quad Q/K tile
        if qh == 0:
            # ---- Q/K loads: one [128, 512] f32 tile per 2 quads; segs
            # s0..s0+7 on partitions 0:64, s0+8..s0+15 on 64:128 ----
            nseg = min(16, 8 * (nq - t))
            qd = qk.tile([128, 512], F32, tag="qd")
            kd = qk.tile([128, 512], F32, tag="kd")
            for h in range(0, nseg, 8):
                nc.sync.dma_start(
                    out=qd[:].rearrange("(h p) f -> h p f", h=2)[
                        h // 8, 0:m
                    ],
                    in_=qv[:, s0 + h : s0 + h + 8, :],
                )
                nc.scalar.dma_start(
                    out=kd[:].rearrange("(h p) f -> h p f", h=2)[
                        h // 8, 0:m
                    ],
                    in_=kv[:, s0 + h : s0 + h + 8, :],
                )
        if qh == 0:
            # ---- V loads: one [128, 520] f32 tile per 2 quads, duo-
            # stacked with a ones column per duo ----
            vd = ld.tile([128, 520], F32, tag="vd")
            vdv = vd[:].rearrange("p (u e) -> p u e", e=65)
            nc.gpsimd.dma_start(
                out=vdv[0:m, :, 0:64], in_=vv[:, s0 : s0 + 16 : 2, :]
            )
            nc.gpsimd.dma_start(
                out=vdv[64 : 64 + m, :, 0:64],
                in_=vv[:, s0 + 1 : s0 + 16 : 2, :],
            )
            nc.vector.memset(vd[:, 64:520:65], 1.0)
        qb = qd[:].bitcast(BF16)[:, 1::2]  # [128, 512] bf16 (truncated)
        kb = kd[:].bitcast(BF16)[:, 1::2]
        vb = vd[:].bitcast(BF16)[:, 1::2]  # [128, 260] bf16

        # ---- transposes: Q^T/K^T duo-stacked [128, m] each ----
        p0q = qh * 64
        qkt_ps = tp.tile([128, 1024], BF16, tag="qkt")  # full 2KB bank
        for j in range(4):
            nc.tensor.transpose(
                qkt_ps[:, j * mp : j * mp + m],
                qb[p0q : p0q + m, j * 128 : (j + 1) * 128],
                identb[p0q : p0q + m, 0:m],
            )
            nc.tensor.transpose(
                qkt_ps[:, (4 + j) * mp : (4 + j) * mp + m],
                kb[p0q : p0q + m, j * 128 : (j + 1) * 128],
                identb[p0q : p0q + m, 0:m],
            )
        qkt = sb.tile([128, 8 * mp], BF16, tag="qkt_s")
        if mp == m:
            nc.vector.tensor_copy(qkt[:], qkt_ps[:, 0 : 8 * mp])
        else:  # strided copy skips the uninitialized pad column per block
            nc.vector.tensor_copy(
                qkt[:].rearrange("p (u x) -> p u x", x=mp)[:, :, 0:m],
                qkt_ps[:, 0 : 8 * mp].rearrange("p (u x) -> p u x", x=mp)[
                    :, :, 0:m
                ],
            )

        # ---- mm1: lt[k, q] per duo-half ----
        lt = ps.tile([128, 512], F32, tag="lt")  # full bank
        for j in range(4):
            qss = qkt[:, j * mp : j * mp + m]
            kss = qkt[:, (4 + j) * mp : (4 + j) * mp + m]
            nc.tensor.matmul(
                lt[0:m, j * m : (j + 1) * m],
                kss[0:64, :],
                qss[0:64, :],
                start=True,
                stop=True,
                tile_position=(0, 0),
            )
            nc.tensor.matmul(
                lt[64 : 64 + m, j * m : (j + 1) * m],
                kss[64:128, :],
                qss[64:128, :],
                start=True,
                stop=True,
                tile_position=(64, 64),
            )

        # ---- softmax numerator (per half: avoid unwritten partitions) ----
        e = sb.tile([128, 4 * mp], BF16, tag="e")
        for p0, p1 in halves:
            if mp == m:
                nc.scalar.activation(
                    e[p0:p1, :],
                    lt[p0:p1, 0 : 4 * m],
                    mybir.ActivationFunctionType.Exp,
                    scale=SCALE,
                )
            else:
                ev = e[p0:p1, :].rearrange("p (u x) -> p u x", x=mp)[:, :, 0:m]
                lv = lt[p0:p1, 0 : 4 * m].rearrange("p (u x) -> p u x", x=m)
                nc.scalar.activation(
                    ev, lv, mybir.ActivationFunctionType.Exp, scale=SCALE
                )

        # ---- mm2: [out_un | s] = e.T @ [V | 1] per duo-half ----
        o_ps = ps.tile([128, 512], F32, tag="ops")  # full bank
        for j in range(4):
            jv = 4 * qh + j
            nc.tensor.matmul(
                o_ps[0:m, j * 65 : (j + 1) * 65],
                e[0:m, j * mp : j * mp + m],
                vb[0:m, jv * 65 : (jv + 1) * 65],
                start=True,
                stop=True,
                tile_position=(0, 0),
            )
            nc.tensor.matmul(
                o_ps[64 : 64 + m, j * 65 : (j + 1) * 65],
                e[64 : 64 + m, j * mp : j * mp + m],
                vb[64 : 64 + m, jv * 65 : (jv + 1) * 65],
                start=True,
                stop=True,
                tile_position=(64, 64),
            )

        # ---- normalize (reciprocal + broadcast multiply) + store ----
        rcp = sb.tile([128, 4], F32, tag="rcp")
        ost = outp.tile([128, 256], F32, tag="ost")
        opsv = o_ps[:, 0:260].rearrange("p (u e) -> p u e", e=65)
        ostv = ost[:].rearrange("p (u e) -> p u e", e=64)
        for p0, p1 in halves:
            nc.vector.reciprocal(rcp[p0:p1, :], o_ps[p0:p1, 64:260:65])
            nc.vector.tensor_mul(
                ostv[p0:p1],
                opsv[p0:p1, :, 0:64],
                rcp[p0:p1, :].unsqueeze(2).to_broadcast([p1 - p0, 4, 64]),
            )
        nc.sync.dma_start(out=ov[:, s0 : s0 + 8 : 2, :], in_=ost[0:m, :])
        nc.scalar.dma_start(
            out=ov[:, s0 + 1 : s0 + 8 : 2, :], in_=ost[64 : 64 + m, :]
        )


def make_pools(tc, stack):
    qk = stack.enter_context(tc.tile_pool(name="qk", bufs=4))
    ld = stack.enter_context(tc.tile_pool(name="ld", bufs=8))
    tp = stack.enter_context(tc.tile_pool(name="tp", bufs=2, space="PSUM"))
    sb = stack.enter_context(tc.tile_pool(name="sb", bufs=4))
    ps = stack.enter_context(tc.tile_pool(name="ps", bufs=2, space="PSUM"))
    outp = stack.enter_context(tc.tile_pool(name="outp", bufs=4))
    return qk, ld, tp, sb, ps, outp


def _build_program():
    nc = bacc.Bacc("TRN2", target_bir_lowering=False, debug=False)
    q = nc.dram_tensor("q", [6, S, D], F32, kind="ExternalInput").ap()
    k = nc.dram_tensor("k", [6, S, D], F32, kind="ExternalInput").ap()
    v = nc.dram_tensor("v", [6, S, D], F32, kind="ExternalInput").ap()
    ident = nc.dram_tensor("ident", [128, 64], F32, kind="ExternalInput").ap()
    o = nc.dram_tensor("o", [6, S, D], F32, kind="ExternalOutput").ap()

    with tile.TileContext(nc) as tc:
        with ExitStack() as stack:
            cpool = stack.enter_context(tc.tile_pool(name="const", bufs=1))
            identt = cpool.tile([128, 64], F32)
            tc.nc.sync.dma_start(out=identt[:], in_=ident)
            identb = identt[:].bitcast(BF16)[:, 1::2]  # [128, 64] bf16
            pools = make_pools(tc, stack)
            for sl, (g, _pair) in enumerate(SLICES):
                build_slice(
                    nc, tc, pools, identb, q[sl], k[sl], v[sl], o[sl], g
                )

    nc.finalize()
    return nc


def _get_program():
    global _PROGRAM
    if _PROGRAM is None:
        _PROGRAM = _build_program()
    return _PROGRAM


def kernel(q, k, v):
    global LAST_RESULT
    q = np.asarray(q, dtype=np.float32)
    k = np.asarray(k, dtype=np.float32)
    v = np.asarray(v, dtype=np.float32)
    assert q.shape == (B, H, S, D), q.shape

    nc = _get_program()

    # identity replicated on both partition halves (for B-half transposes)
    ident = np.zeros((128, 64), np.float32)
    ident[0:64] = np.eye(64, dtype=np.float32)
    ident[64:128] = np.eye(64, dtype=np.float32)

    # (b, head) pair p = b*G + hg within group g; core c owns p in {2c, 2c+1}
    in_maps = []
    for c in range(N_CORES):
        qc = np.empty((6, S, D), np.float32)
        kc = np.empty((6, S, D), np.float32)
        vc = np.empty((6, S, D), np.float32)
        for sl, (g, j) in enumerate(SLICES):
            p = 2 * c + j
            b, hg = p // G, p % G
            head = g * G + hg
            qc[sl] = q[b, head]
            kc[sl] = k[b, head]
            vc[sl] = v[b, head]
        in_maps.append({"q": qc, "k": kc, "v": vc, "ident": ident})

    LAST_RESULT = run_bass_kernel_spmd(nc, in_maps, core_ids=list(range(N_CORES)))

    out = np.zeros((B, H, S, D), np.float32)
    for c in range(N_CORES):
        oc = LAST_RESULT.results[c]["o"]
        for sl, (g, j) in enumerate(SLICES):
            p = 2 * c + j
            b, hg = p // G, p % G
            head = g * G + hg
            out[b, head] = oc[sl]
    return out


# revision 7
# speedup vs baseline: 1.0213x; 1.0213x over previous
"""DilatedAttention Trainium2 kernel (8 NeuronCores, SPMD).

Input  : q, k, v each (2, 24, 8192, 64) float32.
Output : same shape; per head-group windowed attention over dilated
         positions, non-dilated positions zero.

Sharding: 3 head groups x (b in 2, hg in 8) = 16 (b,head) pairs per
group. Core c takes pairs {2c, 2c+1} of every group -> 6 slices per
core, perfectly balanced, no cross-core communication.

Per-core kernel: for each slice, process segments in "quads" (8
segments = 4 duos). A duo packs 2 segments on partition halves:
 - Q/K loaded per 2-quad block (16 segs) as [128, 512] f32 with the
   first 8 segs on partitions 0:64 and the next 8 on 64:128 (spreads
   DMA descriptors over all 16 SDMA engines, 1 HWDGE call each).
 - V loaded duo-stacked [128, 4*65] with a ones column, 1 SWDGE call
   per quad spanning both partition halves.
 - one PE transpose per 128-col chunk yields Q^T/K^T duo-stacked
   [128, m] (B-half chunks transpose from partitions 64:128 via an
   identity replicated on both partition halves).
 - mm1 per half: lt[k,q] = K^T.T @ Q^T   (contraction d=64)
 - exp on ACT (PSUM->SBUF bf16, scale=1/sqrt(d); no max-subtraction
   needed: logits are O(5))
 - mm2 per half: [out_un | s] = e.T @ [V | 1]  (contraction k=m)
 - reciprocal + per-partition broadcast scale on DVE, single merged
   HWDGE store of the dilated rows (ExternalOutput pre-zeroed).

All PSUM tiles are full-bank sized: sub-bank PSUM tiles get packed at
non-bank-aligned offsets, and a matmul output that crosses a PSUM bank
boundary is fatal on hardware.
"""

import sys

if "/opt/trn_rl_repo" not in sys.path:
    sys.path.insert(0, "/opt/trn_rl_repo")

from contextlib import ExitStack

import numpy as np

import concourse.bass as bass  # noqa: F401
import concourse.mybir as mybir
import concourse.tile as tile
from concourse import bacc
from concourse.bass_utils import run_bass_kernel_spmd

B, H, S, D = 2, 24, 8192, 64
W_LIST = [64, 128, 256]
R_LIST = [1, 2, 4]
NG = 3
G = H // NG  # heads per group
N_CORES = 8
SCALE = 1.0 / (D**0.5)

# slice order per core: (group, pair_within_core)
SLICES = [(0, 0), (0, 1), (1, 0), (1, 1), (2, 0), (2, 1)]

F32 = mybir.dt.float32
BF16 = mybir.dt.bfloat16

_PROGRAM = None
LAST_RESULT = None  # BassKernelResults of the most recent run (for test.py)


def build_slice(nc, tc, pools, identb, qs_ap, ks_ap, vs_ap, os_ap, g, nq=None):
    """Emit the program for one (b, head) slice of group g.

    qs_ap.. are [S, D] DRAM APs for this slice.
    """
    qk, ld, tp, sb, ps, outp = pools
    w, r = W_LIST[g], R_LIST[g]
    off = g * r
    m = len(range(off, w, r))  # 64 / 63 / 62
    mp = m + (m & 1)  # even column pitch (PSUM bf16 needs 4B alignment)
    n = qs_ap.shape[0] // w
    if nq is None:
        nq = n // 8
    # partition ranges of the two duo halves; one fused range when m == 64
    halves = [(0, 128)] if m == 64 else [(0, m), (64, 64 + m)]

    qv = qs_ap.rearrange("(n w) d -> w n d", w=w)[off::r]
    kv = ks_ap.rearrange("(n w) d -> w n d", w=w)[off::r]
    vv = vs_ap.rearrange("(n w) d -> w n d", w=w)[off::r]
    ov = os_ap.rearrange("(n w) d -> w n d", w=w)[off::r]

    qd = kd = vd = vb = None
    for t in range(nq):
        s0 = 8 * t
        qh = t & 1  # which partition half of the 2-quad Q/K tile
        if qh == 0:
            # ---- Q/K loads: one [128, 512] f32 tile per 2 quads; segs
            # s0..s0+7 on partitions 0:64, s0+8..s0+15 on 64:128 ----
            nseg = min(16, 8 * (nq - t))
            qd = qk.tile([128, 512], F32, tag="qd")
            kd = qk.tile([128, 512], F32, tag="kd")
            qeng = nc.gpsimd if m == 64 else nc.sync
            keng = nc.gpsimd if m == 64 else nc.scalar
            for h in range(0, nseg, 8):
                qeng.dma_start(
                    out=qd[:].rearrange("(h p) f -> h p f", h=2)[
                        h // 8, 0:m
                    ],
                    in_=qv[:, s0 + h : s0 + h + 8, :],
                )
                keng.dma_start(
                    out=kd[:].rearrange("(h p) f -> h p f", h=2)[
                        h // 8, 0:m
                    ],
                    in_=kv[:, s0 + h : s0 + h + 8, :],
                )
        if qh == 0:
            # ---- V loads: one [128, 520] f32 tile per 2 quads, duo-
            # stacked with a ones column per duo ----
            vd = ld.tile([128, 520], F32, tag="vd")
            vdv = vd[:].rearrange("p (u e) -> p u e", e=65)
            nc.gpsimd.dma_start(
                out=vdv[0:m, :, 0:64], in_=vv[:, s0 : s0 + 16 : 2, :]
            )
            nc.gpsimd.dma_start(
                out=vdv[64 : 64 + m, :, 0:64],
                in_=vv[:, s0 + 1 : s0 + 16 : 2, :],
            )
            nc.vector.memset(vd[:, 64:520:65], 1.0)
        qb = qd[:].bitcast(BF16)[:, 1::2]  # [128, 512] bf16 (truncated)
        kb = kd[:].bitcast(BF16)[:, 1::2]
        vb = vd[:].bitcast(BF16)[:, 1::2]  # [128, 260] bf16

        # ---- transposes: Q^T/K^T duo-stacked [128, m] each ----
        p0q = qh * 64
        qkt_ps = tp.tile([128, 1024], BF16, tag="qkt")  # full 2KB bank
        for j in range(4):
            nc.tensor.transpose(
                qkt_ps[:, j * mp : j * mp + m],
                qb[p0q : p0q + m, j * 128 : (j + 1) * 128],
                identb[p0q : p0q + m, 0:m],
            )
            nc.tensor.transpose(
                qkt_ps[:, (4 + j) * mp : (4 + j) * mp + m],
                kb[p0q : p0q + m, j * 128 : (j + 1) * 128],
                identb[p0q : p0q + m, 0:m],
            )
        qkt = sb.tile([128, 8 * mp], BF16, tag="qkt_s")
        if mp == m:
            nc.vector.tensor_copy(qkt[:], qkt_ps[:, 0 : 8 * mp])
        else:  # strided copy skips the uninitialized pad column per block
            nc.vector.tensor_copy(
                qkt[:].rearrange("p (u x) -> p u x", x=mp)[:, :, 0:m],
                qkt_ps[:, 0 : 8 * mp].rearrange("p (u x) -> p u x", x=mp)[
                    :, :, 0:m
                ],
            )

        # ---- mm1: lt[k, q] per duo-half ----
        lt = ps.tile([128, 512], F32, tag="lt")  # full bank
        for j in range(4):
            qss = qkt[:, j * mp : j * mp + m]
            kss = qkt[:, (4 + j) * mp : (4 + j) * mp + m]
            nc.tensor.matmul(
                lt[0:m, j * m : (j + 1) * m],
                kss[0:64, :],
                qss[0:64, :],
                start=True,
                stop=True,
                tile_position=(0, 0),
            )
            nc.tensor.matmul(
                lt[64 : 64 + m, j * m : (j + 1) * m],
                kss[64:128, :],
                qss[64:128, :],
                start=True,
                stop=True,
                tile_position=(64, 64),
            )

        # ---- softmax numerator (per half: avoid unwritten partitions) ----
        e = sb.tile([128, 4 * mp], BF16, tag="e")
        for p0, p1 in halves:
            if mp == m:
                nc.scalar.activation(
                    e[p0:p1, :],
                    lt[p0:p1, 0 : 4 * m],
                    mybir.ActivationFunctionType.Exp,
                    scale=SCALE,
                )
            else:
                ev = e[p0:p1, :].rearrange("p (u x) -> p u x", x=mp)[:, :, 0:m]
                lv = lt[p0:p1, 0 : 4 * m].rearrange("p (u x) -> p u x", x=m)
                nc.scalar.activation(
                    ev, lv, mybir.ActivationFunctionType.Exp, scale=SCALE
                )

        # ---- mm2: [out_un | s] = e.T @ [V | 1] per duo-half ----
        o_ps = ps.tile([128, 512], F32, tag="ops")  # full bank
        for j in range(4):
            jv = 4 * qh + j
            nc.tensor.matmul(
                o_ps[0:m, j * 65 : (j + 1) * 65],
                e[0:m, j * mp : j * mp + m],
                vb[0:m, jv * 65 : (jv + 1) * 65],
                start=True,
                stop=True,
                tile_position=(0, 0),
            )
            nc.tensor.matmul(
                o_ps[64 : 64 + m, j * 65 : (j + 1) * 65],
                e[64 : 64 + m, j * mp : j * mp + m],
                vb[64 : 64 + m, jv * 65 : (jv + 1) * 65],
                start=True,
                stop=True,
                tile_position=(64, 64),
            )

        # ---- normalize (reciprocal + broadcast multiply) + store ----
        rcp = sb.tile([128, 4], F32, tag="rcp")
        ost = outp.tile([128, 256], F32, tag="ost")
        opsv = o_ps[:, 0:260].rearrange("p (u e) -> p u e", e=65)
        ostv = ost[:].rearrange("p (u e) -> p u e", e=64)
        for p0, p1 in halves:
            nc.vector.reciprocal(rcp[p0:p1, :], o_ps[p0:p1, 64:260:65])
            nc.vector.tensor_mul(
                ostv[p0:p1],
                opsv[p0:p1, :, 0:64],
                rcp[p0:p1, :].unsqueeze(2).to_broadcast([p1 - p0, 4, 64]),
            )
        nc.sync.dma_start(out=ov[:, s0 : s0 + 8 : 2, :], in_=ost[0:m, :])
        nc.scalar.dma_start(
            out=ov[:, s0 + 1 : s0 + 8 : 2, :], in_=ost[64 : 64 + m, :]
        )


def make_pools(tc, stack):
    qk = stack.enter_context(tc.tile_pool(name="qk", bufs=4))
    ld = stack.enter_context(tc.tile_pool(name="ld", bufs=8))
    tp = stack.enter_context(tc.tile_pool(name="tp", bufs=2, space="PSUM"))
    sb = stack.enter_context(tc.tile_pool(name="sb", bufs=4))
    ps = stack.enter_context(tc.tile_pool(name="ps", bufs=2, space="PSUM"))
    outp = stack.enter_context(tc.tile_pool(name="outp", bufs=4))
    return qk, ld, tp, sb, ps, outp


def _build_program():
    nc = bacc.Bacc("TRN2", target_bir_lowering=False, debug=False)
    q = nc.dram_tensor("q", [6, S, D], F32, kind="ExternalInput").ap()
    k = nc.dram_tensor("k", [6, S, D], F32, kind="ExternalInput").ap()
    v = nc.dram_tensor("v", [6, S, D], F32, kind="ExternalInput").ap()
    ident = nc.dram_tensor("ident", [128, 64], F32, kind="ExternalInput").ap()
    o = nc.dram_tensor("o", [6, S, D], F32, kind="ExternalOutput").ap()

    with tile.TileContext(nc) as tc:
        with ExitStack() as stack:
            cpool = stack.enter_context(tc.tile_pool(name="const", bufs=1))
            identt = cpool.tile([128, 64], F32)
            tc.nc.sync.dma_start(out=identt[:], in_=ident)
            identb = identt[:].bitcast(BF16)[:, 1::2]  # [128, 64] bf16
            pools = make_pools(tc, stack)
            for sl, (g, _pair) in enumerate(SLICES):
                build_slice(
                    nc, tc, pools, identb, q[sl], k[sl], v[sl], o[sl], g
                )

    nc.finalize()
    return nc


def _get_program():
    global _PROGRAM
    if _PROGRAM is None:
        _PROGRAM = _build_program()
    return _PROGRAM


def kernel(q, k, v):
    global LAST_RESULT
    q = np.asarray(q, dtype=np.float32)
    k = np.asarray(k, dtype=np.float32)
    v = np.asarray(v, dtype=np.float32)
    assert q.shape == (B, H, S, D), q.shape

    nc = _get_program()

    # identity replicated on both partition halves (for B-half transposes)
    ident = np.zeros((128, 64), np.float32)
    ident[0:64] = np.eye(64, dtype=np.float32)
    ident[64:128] = np.eye(64, dtype=np.float32)

    # (b, head) pair p = b*G + hg within group g; core c owns p in {2c, 2c+1}
    in_maps = []
    for c in range(N_CORES):
        qc = np.empty((6, S, D), np.float32)
        kc = np.empty((6, S, D), np.float32)
        vc = np.empty((6, S, D), np.float32)
        for sl, (g, j) in enumerate(SLICES):
            p = 2 * c + j
            b, hg = p // G, p % G
            head = g * G + hg
            qc[sl] = q[b, head]
            kc[sl] = k[b, head]
            vc[sl] = v[b, head]
        in_maps.append({"q": qc, "k": kc, "v": vc, "ident": ident})

    LAST_RESULT = run_bass_kernel_spmd(nc, in_maps, core_ids=list(range(N_CORES)))

    out = np.zeros((B, H, S, D), np.float32)
    for c in range(N_CORES):
        oc = LAST_RESULT.results[c]["o"]
        for sl, (g, j) in enumerate(SLICES):
            p = 2 * c + j
            b, hg = p // G, p % G
            head = g * G + hg
            out[b, head] = oc[sl]
    return out
